# revision 4
# baseline (speedup 1.0000x reference)
# Multi-head causal attention (B=1, T=4096, D=1024, H=16) on 8 TRN2 NeuronCores.
#
# Sharding: tensor-parallel over heads. Core n computes head channels
# [128n, 128n+128) (= heads 2n, 2n+1), runs the full causal attention for its
# two heads, and produces a full-shape partial output
#   y_n = attn_out[:, ch_n] @ Wo[:, ch_n].T        (4096, 1024)
# The host sums the 8 partials (row-sharded Wo contraction) — no collectives.
#
# Device-side layout (per core):
#   xT   [1024, 4096]  x transposed, bf16 (host-prepped) — contraction on partitions
#   QT/KT [128, 4096]  head channels on partitions (h0: 0-63, h1: 64-127), bf16
#   scoresT[j, i]      keys on partitions, queries on free dim; the softmax sum
#                      over keys rides the PV matmul via a ones-column appended
#                      to V (V' = [V | 1], M=65): psum row 64 = rowsum.
#   exp on ScalarE directly PSUM->SBUF (bf16 out) with 1/sqrt(dk) folded into
#   the activation scale; psum accumulation is always fp32.
#   Causal: only key tiles with j <= i are computed; diagonal 128x128 blocks
#   are masked by a 0/1 upper-triangular multiply after exp.
#
# Pipeline structure: the ScalarE exp stream is the critical resource
# (~1.15 us per 128x1024 ACTIVATE, 144 of them), so the emission order is
# software-pipelined to keep it saturated: at attention step k the kernel
# emits score matmuls for step k+1, the exp for step k, and the PV matmuls
# for step k — so the PE never sits behind an exp it doesn't yet need.
# Scores rotate through a 3-deep pool of 2-bank PSUM tiles (6 banks) with the
# two PV accumulators in the last 2 banks. Projection of chunk c+1 and the
# output-projection tail of chunk c-1 are emitted interleaved into chunk c's
# attention steps so their PE/DVE work fills the PE slack under the exp rate.

import os
import sys

for _p in ("/opt/trn_rl_repo", "/root/.axon_site/_ro/trn_rl_repo"):
    if os.path.isdir(_p) and _p not in sys.path:
        sys.path.insert(0, _p)

import ml_dtypes
import numpy as np

def _ensure_axon_ntff_hook():
    """The agent image's antenv package lacks axon_hooks, which makes
    run_bass_kernel_spmd(trace=True) crash at import under axon. Provide the
    module and register the boot hook so NTFF profiling works."""
    import types

    try:
        import antenv.axon_hooks  # noqa: F401
        return
    except ImportError:
        pass
    try:
        import antenv
    except ImportError:
        return
    mod = types.ModuleType("antenv.axon_hooks")
    mod._hook = None
    mod.set_axon_ntff_profile_hook = lambda h: setattr(mod, "_hook", h)
    mod.get_axon_ntff_profile_hook = lambda: mod._hook
    sys.modules["antenv.axon_hooks"] = mod
    antenv.axon_hooks = mod
    try:
        from trn_agent_boot.trn_boot import _ntff_profile_via_ctypes

        so = "/opt/axon/libaxon_pjrt.so"
        if os.path.exists(so):
            mod._hook = _ntff_profile_via_ctypes(so)
    except Exception:
        pass


_ensure_axon_ntff_hook()

import concourse.bass as bass
import concourse.tile as tile
from concourse import bacc
from concourse import mybir
from concourse.bass_utils import run_bass_kernel_spmd

F32 = mybir.dt.float32
BF16 = mybir.dt.bfloat16
EXP = mybir.ActivationFunctionType.Exp
NPBF = ml_dtypes.bfloat16

D = 1024          # d_model
DK = 64           # head dim
CPC = 128         # channels per core (2 heads)
ICH = 512         # query-chunk size
IH = 512          # i-half width (matmul N / psum bank limit)
JT = 128          # key-tile size

_NC_CACHE = {}


def build(T):
    """Build the per-core Bass program for sequence length T."""
    nc = bacc.Bacc(None, target_bir_lowering=False, debug=False)
    ich = min(ICH, T)
    nch = T // ich

    xT_d = nc.dram_tensor("xT", [D, T], BF16, kind="ExternalInput")
    wqT_d = nc.dram_tensor("wqT", [D, CPC], BF16, kind="ExternalInput")
    wkT_d = nc.dram_tensor("wkT", [D, CPC], BF16, kind="ExternalInput")
    wvT_d = nc.dram_tensor("wvT", [D, CPC], BF16, kind="ExternalInput")
    woT_d = nc.dram_tensor("woT", [CPC, D], BF16, kind="ExternalInput")
    tri_d = nc.dram_tensor("tri", [JT, JT], BF16, kind="ExternalInput")
    ident_d = nc.dram_tensor("ident", [128, 128], BF16, kind="ExternalInput")
    y_d = nc.dram_tensor("y", [T, D], F32, kind="ExternalOutput")
    rs_scratch = nc.dram_tensor("rs_scratch", [nch, 2, ich], F32)

    with tile.TileContext(nc) as tc:
        with (
            tc.tile_pool(name="const", bufs=1) as const,
            tc.tile_pool(name="xtp", bufs=2) as xtp,
            tc.tile_pool(name="vtp", bufs=2) as vtp,
            tc.tile_pool(name="expp", bufs=6) as expp,
            tc.tile_pool(name="outp", bufs=2) as outp,
            tc.tile_pool(name="yp", bufs=4) as yp,
            tc.tile_pool(name="psp", bufs=3, space="PSUM") as psp,
            tc.tile_pool(name="pvp", bufs=1, space="PSUM") as pvp,
        ):
            # ---- constants / persistent state ----
            wq_sb = const.tile([128, D // 128, 128], BF16)
            wk_sb = const.tile([128, D // 128, 128], BF16)
            wv_sb = const.tile([128, D // 128, 128], BF16)
            for w_sb, w_d in ((wq_sb, wqT_d), (wk_sb, wkT_d), (wv_sb, wvT_d)):
                nc.sync.dma_start(
                    out=w_sb, in_=w_d.rearrange("(t p) c -> p t c", p=128)
                )
            wo_sb = const.tile([128, D], BF16)
            nc.sync.dma_start(out=wo_sb, in_=woT_d[:, :])
            tri_sb = const.tile([JT, JT], BF16)
            nc.sync.dma_start(out=tri_sb, in_=tri_d[:, :])
            id_sb = const.tile([128, 128], BF16)
            nc.sync.dma_start(out=id_sb, in_=ident_d[:, :])

            qt_sb = const.tile([128, 2, T], BF16)  # [:,0,:]=QT, [:,1,:]=KT
            # V' = [V_h | 1] per head: [j, jt, 2*65]
            vp_sb = const.tile([128, T // JT, 2 * (DK + 1)], BF16)
            ones_view = vp_sb.rearrange("p t (h c) -> p t h c", h=2)[
                :, :, :, DK : DK + 1
            ]
            nc.vector.memset(ones_view, 1.0)

            xT_v = xT_d.rearrange("(t p) i -> p t i", p=128)

            def gen_proj(c):
                """Generator emitting the Q/K/V projection for chunk c,
                one PE/DVE/DMA op per yield."""
                i0 = c * ich
                xt_ch = xtp.tile([128, D // 128, ich], BF16, tag="xt", name="xt_ch")
                nc.sync.dma_start(
                    out=xt_ch, in_=xT_v[:, :, i0 : i0 + ich]
                )
                yield
                qk_ps = psp.tile([128, 2, ich], F32, tag="ps", name="qk_ps")
                for qk, w_sb in ((0, wq_sb), (1, wk_sb)):
                    for t in range(D // 128):
                        nc.tensor.matmul(
                            out=qk_ps[:, qk, :],
                            lhsT=w_sb[:, t, :],
                            rhs=xt_ch[:, t, :],
                            start=(t == 0),
                            stop=(t == D // 128 - 1),
                        )
                        yield
                nc.vector.tensor_copy(out=qt_sb[:, :, i0 : i0 + ich], in_=qk_ps)
                yield
                vt_ps = psp.tile([128, 2, ich], F32, tag="ps", name="vt_ps")
                for t in range(D // 128):
                    nc.tensor.matmul(
                        out=vt_ps[:, 0, :],
                        lhsT=wv_sb[:, t, :],
                        rhs=xt_ch[:, t, :],
                        start=(t == 0),
                        stop=(t == D // 128 - 1),
                    )
                    yield
                vt_sb = vtp.tile([128, ich], BF16, tag="vt", name="vt_sb")
                nc.vector.tensor_copy(out=vt_sb, in_=vt_ps[:, 0, :])
                yield
                vn_ps = psp.tile([128, ich // 128, 128], BF16, tag="ps", name="vn_ps")
                for sdx in range(ich // 128):
                    nc.tensor.transpose(
                        out=vn_ps[:, sdx, :],
                        in_=vt_sb[:, sdx * 128 : (sdx + 1) * 128],
                        identity=id_sb,
                    )
                    yield
                jt0 = i0 // JT
                nc.vector.tensor_copy(
                    out=vp_sb.rearrange("p t (h c) -> p t h c", h=2)[
                        :, jt0 : jt0 + ich // 128, :, 0:DK
                    ],
                    in_=vn_ps.rearrange("p s (h c) -> p s h c", h=2),
                )
                yield

            def gen_tail_b(c, rsr, outt):
                """Generator: normalization broadcast + output projection for
                chunk c (part B — emitted interleaved into chunk c+1)."""
                i0 = c * ich
                for h in range(2):
                    nc.sync.dma_start(
                        out=rs_scratch[c, h : h + 1, :], in_=rsr[h]
                    )
                yield
                bc = outp.tile([128, ich], F32, tag="bc", name="bc")
                for h in range(2):
                    nc.gpsimd.dma_start(
                        out=bc[h * DK : (h + 1) * DK, :],
                        in_=rs_scratch[c, h : h + 1, :].to_broadcast([DK, ich]),
                    )
                    yield
                nc.vector.tensor_mul(outt, outt, bc)
                yield
                for sidx in range(ich // 128):
                    y_ps = psp.tile([128, 2, IH], F32, tag="ps", name="y_ps")
                    for e in range(2):
                        nc.tensor.matmul(
                            out=y_ps[:, e, :],
                            lhsT=outt[:, sidx * 128 : (sidx + 1) * 128],
                            rhs=wo_sb[:, e * IH : (e + 1) * IH],
                            start=True,
                            stop=True,
                        )
                        yield
                    y_sb = yp.tile([128, D], F32, tag="y", name="y_sb")
                    nc.vector.tensor_copy(
                        out=y_sb, in_=y_ps.rearrange("p a b -> p (a b)")
                    )
                    yield
                    r0 = i0 + sidx * 128
                    nc.sync.dma_start(out=y_d[r0 : r0 + 128, :], in_=y_sb)
                    yield

            def emit_chunk(c, pending):
                """Attention steps for chunk c, draining `pending` generators
                into the step slack. Returns pv psum tiles."""
                i0 = c * ich
                njt = (i0 + ich) // JT
                pv = [
                    pvp.tile([128, ich], F32, tag="pv0", name="pv0"),
                    pvp.tile([128, ich], F32, tag="pv1", name="pv1"),
                ]
                steps = [(p, h) for p in range(njt // 2) for h in range(2)]
                nsteps = len(steps)
                sc_tiles = {}

                def emit_sc(k):
                    p, h = steps[k]
                    hp = slice(h * DK, (h + 1) * DK)
                    sc = psp.tile([128, 2, IH], F32, tag="ps", name="sc")
                    sc_tiles[k] = sc
                    for jj in range(2):
                        jt = 2 * p + jj
                        nc.tensor.matmul(
                            out=sc[:, jj, :],
                            lhsT=qt_sb[hp, 1, jt * JT : (jt + 1) * JT],
                            rhs=qt_sb[hp, 0, i0 : i0 + ich],
                            start=True,
                            stop=True,
                        )

                def emit_exp_pv(k):
                    p, h = steps[k]
                    sc = sc_tiles.pop(k)
                    ex = expp.tile([128, 2, IH], BF16, tag="ex", name="ex")
                    nc.scalar.activation(
                        out=ex, in_=sc, func=EXP, scale=1.0 / np.sqrt(DK)
                    )
                    for jj in range(2):
                        jt = 2 * p + jj
                        off = jt * JT - i0
                        if off >= 0:  # diagonal tile: causal mask
                            if off > 0:
                                nc.vector.memset(ex[:, jj, 0:off], 0.0)
                            nc.vector.tensor_mul(
                                ex[:, jj, off : off + JT],
                                ex[:, jj, off : off + JT],
                                tri_sb,
                            )
                    for jj in range(2):
                        jt = 2 * p + jj
                        nc.tensor.matmul(
                            out=pv[h][0 : DK + 1, :],
                            lhsT=vp_sb[:, jt, h * (DK + 1) : (h + 1) * (DK + 1)],
                            rhs=ex[:, jj, :],
                            start=(jt == 0),
                            stop=(jt == njt - 1),
                        )

                # total interleave ops left (rough count: generators yield once
                # per op); spread evenly over the remaining steps
                def drain(budget):
                    while budget > 0 and pending:
                        try:
                            next(pending[0])
                            budget -= 1
                        except StopIteration:
                            pending.pop(0)

                total_ops = 34 * (1 if c + 1 < nch else 0) + 22 * (1 if c > 0 else 0)
                emit_sc(0)
                for k in range(nsteps):
                    if k + 1 < nsteps:
                        emit_sc(k + 1)
                    emit_exp_pv(k)
                    drain((total_ops + nsteps - 1 - k) // nsteps + 1)
                return pv

            def emit_tail_a(c, pv):
                """Part A of the tail for chunk c: pull the reciprocal rowsums
                and raw attention output out of the pv psum banks (frees them
                for chunk c+1)."""
                rsr = [
                    outp.tile([1, ich], F32, tag=f"rs{h}", name=f"rsr{h}")
                    for h in range(2)
                ]
                for h in range(2):
                    nc.vector.reciprocal(
                        out=rsr[h], in_=pv[h][DK : DK + 1, :]
                    )
                outt = outp.tile([128, ich], BF16, tag="outt", name="outt")
                for h in range(2):
                    nc.vector.tensor_copy(
                        out=outt[h * DK : (h + 1) * DK, :], in_=pv[h][0:DK, :]
                    )
                return rsr, outt

            pending = []
            for _ in gen_proj(0):
                pass
            for c in range(nch):
                if c + 1 < nch:
                    pending.append(gen_proj(c + 1))
                pv = emit_chunk(c, pending)
                rsr, outt = emit_tail_a(c, pv)
                pending.append(gen_tail_b(c, rsr, outt))
            for g in pending:
                for _ in g:
                    pass
    nc.compile()
    return nc


def get_nc(T):
    if T not in _NC_CACHE:
        _NC_CACHE[T] = build(T)
    return _NC_CACHE[T]


TRI = np.triu(np.ones((JT, JT))).astype(NPBF)  # 1 where key j <= query i
IDENT = np.eye(128).astype(NPBF)

LAST_RESULTS = None  # BassKernelResults of the last run (for profiling)


def make_in_maps(x, Wq, Wk, Wv, Wo, n_cores=8):
    """x: (T, D) fp32. Returns per-core input maps (bf16 operands)."""
    xT = np.ascontiguousarray(x.T).astype(NPBF)
    maps = []
    for n in range(n_cores):
        sl = slice(CPC * n, CPC * (n + 1))
        maps.append(
            {
                "xT": xT,
                "wqT": np.ascontiguousarray(Wq[sl, :].T).astype(NPBF),
                "wkT": np.ascontiguousarray(Wk[sl, :].T).astype(NPBF),
                "wvT": np.ascontiguousarray(Wv[sl, :].T).astype(NPBF),
                "woT": np.ascontiguousarray(Wo[:, sl].T).astype(NPBF),
                "tri": TRI,
                "ident": IDENT,
            }
        )
    return maps


def run(x, Wq, Wk, Wv, Wo, T=None, n_cores=8, trace=False):
    global LAST_RESULTS
    T = T if T is not None else x.shape[0]
    nc = get_nc(T)
    in_maps = make_in_maps(x, Wq, Wk, Wv, Wo, n_cores)
    res = run_bass_kernel_spmd(
        nc, in_maps, core_ids=list(range(n_cores)), trace=trace
    )
    LAST_RESULTS = res
    y = np.zeros((T, D), dtype=np.float64)
    for r in res.results:
        y += r["y"].astype(np.float64)
    return y.astype(np.float32)


def kernel(x, Wq, Wk, Wv, Wo):
    x = np.asarray(x, dtype=np.float32)
    B, T, _ = x.shape
    trace = bool(os.environ.get("MHA_TRACE"))
    y = run(
        np.ascontiguousarray(x.reshape(T, D)),
        np.asarray(Wq, np.float32),
        np.asarray(Wk, np.float32),
        np.asarray(Wv, np.float32),
        np.asarray(Wo, np.float32),
        T=T,
        trace=trace,
    )
    if trace and LAST_RESULTS is not None and LAST_RESULTS.exec_time_ns:
        print(f"HW exec time: {LAST_RESULTS.exec_time_ns} ns")
    return y.reshape(B, T, D)


# revision 9
# speedup vs baseline: 1.1101x; 1.1101x over previous
# Multi-head causal attention (B=1, T=4096, D=1024, H=16) on 8 TRN2 NeuronCores.
#
# Sharding: tensor-parallel over heads. Core n computes head channels
# [128n, 128n+128) (= heads 2n, 2n+1), runs the full causal attention for its
# two heads, and produces a full-shape partial output
#   y_n = attn_out[:, ch_n] @ Wo[:, ch_n].T        (4096, 1024)
# The host sums the 8 partials (row-sharded Wo contraction) — no collectives.
#
# Device-side layout (per core):
#   xT   [1024, 4096]  x transposed, bf16 (host-prepped) — contraction on partitions
#   QT/KT [128, 4096]  head channels on partitions (h0: 0-63, h1: 64-127), bf16
#   scoresT[j, i]      keys on partitions, queries on free dim; the softmax sum
#                      over keys rides the PV matmul via a ones-column appended
#                      to V (V' = [V | 1], M=65): psum row 64 = rowsum.
#   exp on ScalarE directly PSUM->SBUF (bf16 out) with 1/sqrt(dk) folded into
#   the activation scale; psum accumulation is always fp32.
#   Causal: only key tiles with j <= i are computed; diagonal 128x128 blocks
#   are masked by a 0/1 upper-triangular multiply after exp.
#
# Pipeline structure: the ScalarE exp stream is the critical resource
# (~1.15 us per 128x1024 ACTIVATE, 144 of them), so the emission order is
# software-pipelined to keep it saturated: at attention step k the kernel
# emits score matmuls for step k+1, the exp for step k, and the PV matmuls
# for step k — so the PE never sits behind an exp it doesn't yet need.
# Scores rotate through a 3-deep pool of 2-bank PSUM tiles (6 banks) with the
# two PV accumulators in the last 2 banks. Projection of chunk c+1 and the
# output-projection tail of chunk c-1 are emitted interleaved into chunk c's
# attention steps so their PE/DVE work fills the PE slack under the exp rate.

import os
import sys

for _p in ("/opt/trn_rl_repo", "/root/.axon_site/_ro/trn_rl_repo"):
    if os.path.isdir(_p) and _p not in sys.path:
        sys.path.insert(0, _p)

import ml_dtypes
import numpy as np

def _ensure_axon_ntff_hook():
    """The agent image's antenv package lacks axon_hooks, which makes
    run_bass_kernel_spmd(trace=True) crash at import under axon. Provide the
    module and register the boot hook so NTFF profiling works."""
    import types

    try:
        import antenv.axon_hooks  # noqa: F401
        return
    except ImportError:
        pass
    try:
        import antenv
    except ImportError:
        return
    mod = types.ModuleType("antenv.axon_hooks")
    mod._hook = None
    mod.set_axon_ntff_profile_hook = lambda h: setattr(mod, "_hook", h)
    mod.get_axon_ntff_profile_hook = lambda: mod._hook
    sys.modules["antenv.axon_hooks"] = mod
    antenv.axon_hooks = mod
    try:
        from trn_agent_boot.trn_boot import _ntff_profile_via_ctypes

        so = "/opt/axon/libaxon_pjrt.so"
        if os.path.exists(so):
            mod._hook = _ntff_profile_via_ctypes(so)
    except Exception:
        pass


_ensure_axon_ntff_hook()

import concourse.bass as bass
import concourse.tile as tile
from concourse import bacc
from concourse import mybir
from concourse.bass_utils import run_bass_kernel_spmd

F32 = mybir.dt.float32
BF16 = mybir.dt.bfloat16
EXP = mybir.ActivationFunctionType.Exp
NPBF = ml_dtypes.bfloat16

D = 1024          # d_model
DK = 64           # head dim
CPC = 128         # channels per core (2 heads)
ICH = 512         # query-chunk size
IH = 512          # i-half width (matmul N / psum bank limit)
JT = 128          # key-tile size

_NC_CACHE = {}


def build(T):
    """Build the per-core Bass program for sequence length T."""
    nc = bacc.Bacc(None, target_bir_lowering=False, debug=False)
    ich = min(ICH, T)
    nch = T // ich

    xT_d = nc.dram_tensor("xT", [D, T], BF16, kind="ExternalInput")
    wqT_d = nc.dram_tensor("wqT", [D, CPC], BF16, kind="ExternalInput")
    wkT_d = nc.dram_tensor("wkT", [D, CPC], BF16, kind="ExternalInput")
    wvT_d = nc.dram_tensor("wvT", [D, CPC], BF16, kind="ExternalInput")
    woT_d = nc.dram_tensor("woT", [CPC, D], BF16, kind="ExternalInput")
    tri_d = nc.dram_tensor("tri", [JT, JT], BF16, kind="ExternalInput")
    ident_d = nc.dram_tensor("ident", [128, 128], BF16, kind="ExternalInput")
    y_d = nc.dram_tensor("y", [T, D], F32, kind="ExternalOutput")
    rs_scratch = nc.dram_tensor("rs_scratch", [nch, 2, ich], F32)
    rs2_scratch = nc.dram_tensor("rs2_scratch", [nch, 2 * ich], F32)

    with tile.TileContext(nc) as tc:
        with (
            tc.tile_pool(name="const", bufs=1) as const,
            tc.tile_pool(name="xtp", bufs=2) as xtp,
            tc.tile_pool(name="vtp", bufs=2) as vtp,
            tc.tile_pool(name="expp", bufs=6) as expp,
            tc.tile_pool(name="outp", bufs=2) as outp,
            tc.tile_pool(name="yp", bufs=4) as yp,
            tc.tile_pool(name="psp", bufs=3, space="PSUM") as psp,
            tc.tile_pool(name="pvp", bufs=1, space="PSUM") as pvp,
        ):
            # ---- constants / persistent state ----
            wq_sb = const.tile([128, D // 128, 128], BF16)
            wk_sb = const.tile([128, D // 128, 128], BF16)
            wv_sb = const.tile([128, D // 128, 128], BF16)
            for w_sb, w_d in ((wq_sb, wqT_d), (wk_sb, wkT_d), (wv_sb, wvT_d)):
                nc.sync.dma_start(
                    out=w_sb, in_=w_d.rearrange("(t p) c -> p t c", p=128)
                )
            wo_sb = const.tile([128, D], BF16)
            nc.sync.dma_start(out=wo_sb, in_=woT_d[:, :])
            tri_sb = const.tile([JT, JT], BF16)
            nc.sync.dma_start(out=tri_sb, in_=tri_d[:, :])
            id_sb = const.tile([128, 128], BF16)
            nc.sync.dma_start(out=id_sb, in_=ident_d[:, :])

            qt_sb = const.tile([128, 2, T], BF16)  # [:,0,:]=QT, [:,1,:]=KT
            # V' = [V_h | 1] per head: [j, jt, 2*65]
            vp_sb = const.tile([128, T // JT, 2 * (DK + 1)], BF16)
            ones_view = vp_sb.rearrange("p t (h c) -> p t h c", h=2)[
                :, :, :, DK : DK + 1
            ]
            nc.vector.memset(ones_view, 1.0)

            xT_v = xT_d.rearrange("(t p) i -> p t i", p=128)

            def gen_proj(c):
                """Generator emitting the Q/K/V projection for chunk c,
                one PE/DVE/DMA op per yield."""
                i0 = c * ich
                xt_ch = xtp.tile([128, D // 128, ich], BF16, tag="xt", name="xt_ch")
                nc.sync.dma_start(
                    out=xt_ch, in_=xT_v[:, :, i0 : i0 + ich]
                )
                yield
                qk_ps = psp.tile([128, 2, ich], F32, tag="ps", name="qk_ps")
                for qk, w_sb in ((0, wq_sb), (1, wk_sb)):
                    for t in range(D // 128):
                        nc.tensor.matmul(
                            out=qk_ps[:, qk, :],
                            lhsT=w_sb[:, t, :],
                            rhs=xt_ch[:, t, :],
                            start=(t == 0),
                            stop=(t == D // 128 - 1),
                        )
                        yield
                nc.vector.tensor_copy(out=qt_sb[:, :, i0 : i0 + ich], in_=qk_ps)
                yield
                vt_ps = psp.tile([128, 2, ich], F32, tag="ps", name="vt_ps")
                for t in range(D // 128):
                    nc.tensor.matmul(
                        out=vt_ps[:, 0, :],
                        lhsT=wv_sb[:, t, :],
                        rhs=xt_ch[:, t, :],
                        start=(t == 0),
                        stop=(t == D // 128 - 1),
                    )
                    yield
                vt_sb = vtp.tile([128, ich], BF16, tag="vt", name="vt_sb")
                nc.vector.tensor_copy(out=vt_sb, in_=vt_ps[:, 0, :])
                yield
                vn_ps = psp.tile([128, ich // 128, 128], BF16, tag="ps", name="vn_ps")
                for sdx in range(ich // 128):
                    nc.tensor.transpose(
                        out=vn_ps[:, sdx, :],
                        in_=vt_sb[:, sdx * 128 : (sdx + 1) * 128],
                        identity=id_sb,
                    )
                    yield
                jt0 = i0 // JT
                nc.vector.tensor_copy(
                    out=vp_sb.rearrange("p t (h c) -> p t h c", h=2)[
                        :, jt0 : jt0 + ich // 128, :, 0:DK
                    ],
                    in_=vn_ps.rearrange("p s (h c) -> p s h c", h=2),
                )
                yield

            def gen_tail(c, pv):
                """Generator: rowsum reciprocal (spread across partitions via
                a scratch roundtrip — a 1-partition DVE reciprocal would cost
                3.3us), normalization, and output projection for chunk c.
                The first block frees the pv psum banks for chunk c+1."""
                i0 = c * ich
                rs_sb = outp.tile([1, 2, ich], F32, tag="rs", name="rs_sb")
                for h in range(2):
                    nc.vector.tensor_copy(
                        out=rs_sb[0:1, h, :], in_=pv[h][DK : DK + 1, :]
                    )
                outt = outp.tile([128, ich], BF16, tag="outt", name="outt")
                for h in range(2):
                    nc.vector.tensor_copy(
                        out=outt[h * DK : (h + 1) * DK, :], in_=pv[h][0:DK, :]
                    )
                yield
                nc.sync.dma_start(out=rs_scratch[c : c + 1, :, :], in_=rs_sb)
                yield
                rsp = outp.tile([128, 2 * ich // 128], F32, tag="rsp", name="rsp")
                nc.sync.dma_start(
                    out=rsp,
                    in_=rs_scratch[c].rearrange("h i -> (h i)").rearrange(
                        "(p f) -> p f", p=128
                    ),
                )
                yield
                nc.vector.reciprocal(out=rsp, in_=rsp)
                yield
                nc.sync.dma_start(
                    out=rs2_scratch[c].rearrange("(p f) -> p f", p=128), in_=rsp
                )
                yield
                bc = outp.tile([128, ich], F32, tag="bc", name="bc")
                for h in range(2):
                    nc.gpsimd.dma_start(
                        out=bc[h * DK : (h + 1) * DK, :],
                        in_=rs2_scratch[c].rearrange("(h i) -> h i", h=2)[
                            h : h + 1, :
                        ].to_broadcast([DK, ich]),
                    )
                    yield
                nc.vector.tensor_mul(outt, outt, bc)
                yield
                for sidx in range(ich // 128):
                    y_ps = psp.tile([128, 2, IH], F32, tag="ps", name="y_ps")
                    for e in range(2):
                        nc.tensor.matmul(
                            out=y_ps[:, e, :],
                            lhsT=outt[:, sidx * 128 : (sidx + 1) * 128],
                            rhs=wo_sb[:, e * IH : (e + 1) * IH],
                            start=True,
                            stop=True,
                        )
                        yield
                    y_sb = yp.tile([128, D], F32, tag="y", name="y_sb")
                    nc.vector.tensor_copy(
                        out=y_sb, in_=y_ps.rearrange("p a b -> p (a b)")
                    )
                    yield
                    r0 = i0 + sidx * 128
                    nc.sync.dma_start(out=y_d[r0 : r0 + 128, :], in_=y_sb)
                    yield

            def emit_chunk(c, pending):
                """Attention steps for chunk c, draining `pending` generators
                into the step slack. Returns pv psum tiles."""
                i0 = c * ich
                njt = (i0 + ich) // JT
                pv = [
                    pvp.tile([128, ich], F32, tag="pv0", name="pv0"),
                    pvp.tile([128, ich], F32, tag="pv1", name="pv1"),
                ]
                steps = [(p, h) for p in range(njt // 2) for h in range(2)]
                nsteps = len(steps)
                sc_tiles = {}

                def emit_sc(k):
                    p, h = steps[k]
                    hp = slice(h * DK, (h + 1) * DK)
                    sc = psp.tile([128, 2, IH], F32, tag="ps", name="sc")
                    sc_tiles[k] = sc
                    for jj in range(2):
                        jt = 2 * p + jj
                        nc.tensor.matmul(
                            out=sc[:, jj, :],
                            lhsT=qt_sb[hp, 1, jt * JT : (jt + 1) * JT],
                            rhs=qt_sb[hp, 0, i0 : i0 + ich],
                            start=True,
                            stop=True,
                        )

                ex_tiles = {}

                def emit_exp(k):
                    p, h = steps[k]
                    sc = sc_tiles.pop(k)
                    ex = expp.tile([128, 2, IH], BF16, tag="ex", name="ex")
                    ex_tiles[k] = ex
                    nc.scalar.activation(
                        out=ex, in_=sc, func=EXP, scale=1.0 / np.sqrt(DK)
                    )
                    for jj in range(2):
                        jt = 2 * p + jj
                        off = jt * JT - i0
                        if off >= 0:  # diagonal tile: causal mask
                            if off > 0:
                                nc.vector.memset(ex[:, jj, 0:off], 0.0)
                            nc.vector.tensor_mul(
                                ex[:, jj, off : off + JT],
                                ex[:, jj, off : off + JT],
                                tri_sb,
                            )

                def emit_pv(k):
                    p, h = steps[k]
                    ex = ex_tiles.pop(k)
                    for jj in range(2):
                        jt = 2 * p + jj
                        nc.tensor.matmul(
                            out=pv[h][0 : DK + 1, :],
                            lhsT=vp_sb[:, jt, h * (DK + 1) : (h + 1) * (DK + 1)],
                            rhs=ex[:, jj, :],
                            start=(jt == 0),
                            stop=(jt == njt - 1),
                        )

                # drain pending generators round-robin so the tail's serial
                # DMA chain starts early in the chunk instead of bunching at
                # the end
                def drain(budget):
                    while budget > 0 and pending:
                        progressed = False
                        for g in list(pending):
                            if budget <= 0:
                                break
                            try:
                                next(g)
                                budget -= 1
                                progressed = True
                            except StopIteration:
                                pending.remove(g)
                        if not progressed and not pending:
                            break

                total_ops = 34 * (1 if c + 1 < nch else 0) + 26 * (1 if c > 0 else 0)
                # software pipeline: at step k emit scores(k+1), exp(k),
                # PV(k-2) — PV lags 2 steps so psum-handoff waits at chunk
                # boundaries stay off the PE critical path
                emit_sc(0)
                for k in range(nsteps):
                    if k + 1 < nsteps:
                        emit_sc(k + 1)
                    emit_exp(k)
                    if k - 2 >= 0:
                        emit_pv(k - 2)
                    drain((total_ops + nsteps - 1 - k) // nsteps + 1)
                emit_pv(nsteps - 2)
                emit_pv(nsteps - 1)
                return pv

            pending = []
            for _ in gen_proj(0):
                pass
            for c in range(nch):
                if c + 1 < nch:
                    pending.append(gen_proj(c + 1))
                pv = emit_chunk(c, pending)
                pending.append(gen_tail(c, pv))
            for g in pending:
                for _ in g:
                    pass
    nc.compile()
    return nc


def get_nc(T):
    if T not in _NC_CACHE:
        _NC_CACHE[T] = build(T)
    return _NC_CACHE[T]


TRI = np.triu(np.ones((JT, JT))).astype(NPBF)  # 1 where key j <= query i
IDENT = np.eye(128).astype(NPBF)

LAST_RESULTS = None  # BassKernelResults of the last run (for profiling)


def make_in_maps(x, Wq, Wk, Wv, Wo, n_cores=8):
    """x: (T, D) fp32. Returns per-core input maps (bf16 operands)."""
    xT = np.ascontiguousarray(x.T).astype(NPBF)
    maps = []
    for n in range(n_cores):
        sl = slice(CPC * n, CPC * (n + 1))
        maps.append(
            {
                "xT": xT,
                "wqT": np.ascontiguousarray(Wq[sl, :].T).astype(NPBF),
                "wkT": np.ascontiguousarray(Wk[sl, :].T).astype(NPBF),
                "wvT": np.ascontiguousarray(Wv[sl, :].T).astype(NPBF),
                "woT": np.ascontiguousarray(Wo[:, sl].T).astype(NPBF),
                "tri": TRI,
                "ident": IDENT,
            }
        )
    return maps


def run(x, Wq, Wk, Wv, Wo, T=None, n_cores=8, trace=False):
    global LAST_RESULTS
    T = T if T is not None else x.shape[0]
    nc = get_nc(T)
    in_maps = make_in_maps(x, Wq, Wk, Wv, Wo, n_cores)
    res = run_bass_kernel_spmd(
        nc, in_maps, core_ids=list(range(n_cores)), trace=trace
    )
    LAST_RESULTS = res
    y = np.zeros((T, D), dtype=np.float64)
    for r in res.results:
        y += r["y"].astype(np.float64)
    return y.astype(np.float32)


def kernel(x, Wq, Wk, Wv, Wo):
    x = np.asarray(x, dtype=np.float32)
    B, T, _ = x.shape
    trace = bool(os.environ.get("MHA_TRACE"))
    y = run(
        np.ascontiguousarray(x.reshape(T, D)),
        np.asarray(Wq, np.float32),
        np.asarray(Wk, np.float32),
        np.asarray(Wv, np.float32),
        np.asarray(Wo, np.float32),
        T=T,
        trace=trace,
    )
    if trace and LAST_RESULTS is not None and LAST_RESULTS.exec_time_ns:
        print(f"HW exec time: {LAST_RESULTS.exec_time_ns} ns")
    return y.reshape(B, T, D)


# revision 10
# speedup vs baseline: 1.2625x; 1.1373x over previous
# Multi-head causal attention (B=1, T=4096, D=1024, H=16) on 8 TRN2 NeuronCores.
#
# Sharding: tensor-parallel over heads. Core n computes head channels
# [128n, 128n+128) (= heads 2n, 2n+1), runs the full causal attention for its
# two heads, and produces a full-shape partial output
#   y_n = attn_out[:, ch_n] @ Wo[:, ch_n].T        (4096, 1024)
# The host sums the 8 partials (row-sharded Wo contraction) — no collectives.
#
# Device-side layout (per core):
#   xT   [128, 8, T]   x transposed + partition-tiled on the host so the DMA
#                      is contiguous; contraction (d_model) on partitions.
#   QT/KT [128, 2, T]  head channels on partitions (h0: 0-63, h1: 64-127), bf16
#   scoresT[j, i]      keys on partitions, queries on free dim; the softmax sum
#                      over keys rides the PV matmul via a ones-column appended
#                      to V (V' = [V | 1], M=65): psum row 64 = rowsum.
#   exp on ScalarE directly PSUM->SBUF (bf16 out) with 1/sqrt(dk) folded into
#   the activation scale; psum accumulation is always fp32.
#   Causal: only key tiles with j <= i are computed; diagonal 128x128 blocks
#   are masked by a 0/1 upper-triangular multiply after exp.
#
# Schedule: the ScalarE exp stream (144 ACTIVATEs of 128x1024, ~1.15us each)
# is the critical resource; everything is arranged to keep it saturated.
# Attention is a flat sequence of (key-tile-pair, head) steps per 512-query
# chunk; at step k the kernel emits score matmuls for step k+1, the exp for
# step k, and PV matmuls for step k-2 (the 2-step lag keeps psum-handoff
# waits at chunk boundaries off the in-order PE queue's critical path).
# PSUM banks: 2 rotating 2-bank score tiles (4), one 2-bank aux slot for
# projection/output-projection tiles (2), two 1-bank PV accumulators (2).
# The Q/K/V projection of chunk c+1 and the normalization+output projection
# tail of chunk c-1 are emitted interleaved into chunk c's attention steps;
# the tail's PE section (y matmuls) is gated to the last 45% of the chunk so
# the rowsum-reciprocal DMA roundtrip it depends on has completed.

import os
import sys

for _p in ("/opt/trn_rl_repo", "/root/.axon_site/_ro/trn_rl_repo"):
    if os.path.isdir(_p) and _p not in sys.path:
        sys.path.insert(0, _p)

import ml_dtypes
import numpy as np

def _ensure_axon_ntff_hook():
    """The agent image's antenv package lacks axon_hooks, which makes
    run_bass_kernel_spmd(trace=True) crash at import under axon. Provide the
    module and register the boot hook so NTFF profiling works."""
    import types

    try:
        import antenv.axon_hooks  # noqa: F401
        return
    except ImportError:
        pass
    try:
        import antenv
    except ImportError:
        return
    mod = types.ModuleType("antenv.axon_hooks")
    mod._hook = None
    mod.set_axon_ntff_profile_hook = lambda h: setattr(mod, "_hook", h)
    mod.get_axon_ntff_profile_hook = lambda: mod._hook
    sys.modules["antenv.axon_hooks"] = mod
    antenv.axon_hooks = mod
    try:
        from trn_agent_boot.trn_boot import _ntff_profile_via_ctypes

        so = "/opt/axon/libaxon_pjrt.so"
        if os.path.exists(so):
            mod._hook = _ntff_profile_via_ctypes(so)
    except Exception:
        pass


_ensure_axon_ntff_hook()

import concourse.bass as bass
import concourse.tile as tile
from concourse import bacc
from concourse import mybir
from concourse.bass_utils import run_bass_kernel_spmd

F32 = mybir.dt.float32
BF16 = mybir.dt.bfloat16
EXP = mybir.ActivationFunctionType.Exp
NPBF = ml_dtypes.bfloat16

D = 1024          # d_model
DK = 64           # head dim
CPC = 128         # channels per core (2 heads)
ICH = 512         # query-chunk size
IH = 512          # i-half width (matmul N / psum bank limit)
JT = 128          # key-tile size
NT = D // 128     # d_model tiles

_NC_CACHE = {}


def build(T):
    """Build the per-core Bass program for sequence length T."""
    nc = bacc.Bacc(None, target_bir_lowering=False, debug=False)
    ich = min(ICH, T)
    nch = T // ich

    xT_d = nc.dram_tensor("xT", [128, NT, T], BF16, kind="ExternalInput")
    wqT_d = nc.dram_tensor("wqT", [128, NT, CPC], BF16, kind="ExternalInput")
    wkT_d = nc.dram_tensor("wkT", [128, NT, CPC], BF16, kind="ExternalInput")
    wvT_d = nc.dram_tensor("wvT", [128, NT, CPC], BF16, kind="ExternalInput")
    woT_d = nc.dram_tensor("woT", [CPC, D], BF16, kind="ExternalInput")
    tri_d = nc.dram_tensor("tri", [JT, JT], BF16, kind="ExternalInput")
    ident_d = nc.dram_tensor("ident", [128, 128], BF16, kind="ExternalInput")
    y_d = nc.dram_tensor("y", [T, D], F32, kind="ExternalOutput")
    rs_scratch = nc.dram_tensor("rs_scratch", [nch, 2, ich], F32)
    rs2_scratch = nc.dram_tensor("rs2_scratch", [nch, 2 * ich], F32)

    with tile.TileContext(nc) as tc:
        with (
            tc.tile_pool(name="const", bufs=1) as const,
            tc.tile_pool(name="xtp", bufs=2) as xtp,
            tc.tile_pool(name="vtp", bufs=2) as vtp,
            tc.tile_pool(name="expp", bufs=6) as expp,
            tc.tile_pool(name="outp", bufs=2) as outp,
            tc.tile_pool(name="yp", bufs=4) as yp,
            tc.tile_pool(name="psp", bufs=2, space="PSUM") as psp,
            tc.tile_pool(name="pvp", bufs=1, space="PSUM") as pvp,
        ):
            # ---- warm the ScalarE exp table while the weight DMAs run ----
            warm = const.tile([128, 1], F32)
            nc.vector.memset(warm, 0.0)
            nc.scalar.activation(out=warm, in_=warm, func=EXP)

            # ---- constants / persistent state ----
            wq_sb = const.tile([128, NT, 128], BF16)
            wk_sb = const.tile([128, NT, 128], BF16)
            wv_sb = const.tile([128, NT, 128], BF16)
            for w_sb, w_d in ((wq_sb, wqT_d), (wk_sb, wkT_d), (wv_sb, wvT_d)):
                nc.sync.dma_start(out=w_sb, in_=w_d[:, :, :])
            wo_sb = const.tile([128, D], BF16)
            nc.sync.dma_start(out=wo_sb, in_=woT_d[:, :])
            tri_sb = const.tile([JT, JT], BF16)
            nc.sync.dma_start(out=tri_sb, in_=tri_d[:, :])
            id_sb = const.tile([128, 128], BF16)
            nc.sync.dma_start(out=id_sb, in_=ident_d[:, :])

            qt_sb = const.tile([128, 2, T], BF16)  # [:,0,:]=QT, [:,1,:]=KT
            # V' = [V_h | 1] per head: [j, jt, 2*65]
            vp_sb = const.tile([128, T // JT, 2 * (DK + 1)], BF16)
            ones_view = vp_sb.rearrange("p t (h c) -> p t h c", h=2)[
                :, :, :, DK : DK + 1
            ]
            nc.vector.memset(ones_view, 1.0)

            xt_tiles = {}

            def prefetch_xt(c):
                xt_ch = xtp.tile([128, NT, ich], BF16, tag="xt", name="xt_ch")
                nc.sync.dma_start(
                    out=xt_ch, in_=xT_d[:, :, c * ich : (c + 1) * ich]
                )
                xt_tiles[c] = xt_ch

            def gen_proj(c):
                """Generator emitting the Q/K/V projection for chunk c,
                one PE/DVE op per yield (the x chunk was prefetched)."""
                i0 = c * ich
                xt_ch = xt_tiles.pop(c)
                qk_ps = psp.tile([128, 2, ich], F32, tag="aux", bufs=1, name="qk_ps")
                for qk, w_sb in ((0, wq_sb), (1, wk_sb)):
                    for t in range(NT):
                        nc.tensor.matmul(
                            out=qk_ps[:, qk, :],
                            lhsT=w_sb[:, t, :],
                            rhs=xt_ch[:, t, :],
                            start=(t == 0),
                            stop=(t == NT - 1),
                        )
                        yield
                nc.vector.tensor_copy(out=qt_sb[:, :, i0 : i0 + ich], in_=qk_ps)
                yield
                vt_ps = psp.tile([128, 2, ich], F32, tag="aux", bufs=1, name="vt_ps")
                for t in range(NT):
                    nc.tensor.matmul(
                        out=vt_ps[:, 0, :],
                        lhsT=wv_sb[:, t, :],
                        rhs=xt_ch[:, t, :],
                        start=(t == 0),
                        stop=(t == NT - 1),
                    )
                    yield
                vt_sb = vtp.tile([128, ich], BF16, tag="vt", name="vt_sb")
                nc.vector.tensor_copy(out=vt_sb, in_=vt_ps[:, 0, :])
                yield
                vn_ps = psp.tile(
                    [128, ich // 128, 128], BF16, tag="aux", bufs=1, name="vn_ps"
                )
                for sdx in range(ich // 128):
                    nc.tensor.transpose(
                        out=vn_ps[:, sdx, :],
                        in_=vt_sb[:, sdx * 128 : (sdx + 1) * 128],
                        identity=id_sb,
                    )
                    yield
                jt0 = i0 // JT
                nc.vector.tensor_copy(
                    out=vp_sb.rearrange("p t (h c) -> p t h c", h=2)[
                        :, jt0 : jt0 + ich // 128, :, 0:DK
                    ],
                    in_=vn_ps.rearrange("p s (h c) -> p s h c", h=2),
                )
                yield

            def gen_tail_head(c, pv, out_slot):
                """Rowsum + raw-output extraction (frees the pv psum banks),
                then the reciprocal DMA roundtrip: rowsums scattered across
                128 partitions for the DVE reciprocal (a 1-partition
                reciprocal would cost 3.3us), then broadcast back."""
                rs_sb = outp.tile([1, 2, ich], F32, tag="rs", name="rs_sb")
                for h in range(2):
                    nc.vector.tensor_copy(
                        out=rs_sb[0:1, h, :], in_=pv[h][DK : DK + 1, :]
                    )
                outt = outp.tile([128, ich], BF16, tag="outt", name="outt")
                for h in range(2):
                    nc.vector.tensor_copy(
                        out=outt[h * DK : (h + 1) * DK, :], in_=pv[h][0:DK, :]
                    )
                out_slot["outt"] = outt
                yield
                nc.sync.dma_start(out=rs_scratch[c : c + 1, :, :], in_=rs_sb)
                yield
                rsp = outp.tile([128, 2 * ich // 128], F32, tag="rsp", name="rsp")
                nc.sync.dma_start(
                    out=rsp,
                    in_=rs_scratch[c].rearrange("h i -> (h i)").rearrange(
                        "(p f) -> p f", p=128
                    ),
                )
                yield
                nc.vector.reciprocal(out=rsp, in_=rsp)
                yield
                nc.sync.dma_start(
                    out=rs2_scratch[c].rearrange("(p f) -> p f", p=128), in_=rsp
                )
                yield
                bc = outp.tile([128, ich], F32, tag="bc", name="bc")
                for h in range(2):
                    nc.gpsimd.dma_start(
                        out=bc[h * DK : (h + 1) * DK, :],
                        in_=rs2_scratch[c].rearrange("(h i) -> h i", h=2)[
                            h : h + 1, :
                        ].to_broadcast([DK, ich]),
                    )
                    yield
                out_slot["bc"] = bc

            def gen_tail_y(c, out_slot):
                """Normalize and project: gated to late in the next chunk so
                the y matmuls never head-of-line block the PE queue on the
                reciprocal roundtrip."""
                i0 = c * ich
                outt, bc = out_slot["outt"], out_slot["bc"]
                nc.vector.tensor_mul(outt, outt, bc)
                yield
                for sidx in range(ich // 128):
                    y_ps = psp.tile(
                        [128, 2, IH], F32, tag="aux", bufs=1, name="y_ps"
                    )
                    for e in range(2):
                        nc.tensor.matmul(
                            out=y_ps[:, e, :],
                            lhsT=outt[:, sidx * 128 : (sidx + 1) * 128],
                            rhs=wo_sb[:, e * IH : (e + 1) * IH],
                            start=True,
                            stop=True,
                        )
                        yield
                    y_sb = yp.tile([128, D], F32, tag="y", name="y_sb")
                    nc.vector.tensor_copy(
                        out=y_sb, in_=y_ps.rearrange("p a b -> p (a b)")
                    )
                    yield
                    r0 = i0 + sidx * 128
                    nc.sync.dma_start(out=y_d[r0 : r0 + 128, :], in_=y_sb)
                    yield

            def emit_chunk(c, pending):
                """Attention steps for chunk c, draining `pending` generator
                entries [min_frac, gen] into the step slack."""
                i0 = c * ich
                njt = (i0 + ich) // JT
                pv = [
                    pvp.tile([128, ich], F32, tag="pv0", name="pv0"),
                    pvp.tile([128, ich], F32, tag="pv1", name="pv1"),
                ]
                steps = [(p, h) for p in range(njt // 2) for h in range(2)]
                nsteps = len(steps)
                sc_tiles = {}
                ex_tiles = {}

                def emit_sc(k):
                    p, h = steps[k]
                    hp = slice(h * DK, (h + 1) * DK)
                    sc = psp.tile([128, 2, IH], F32, tag="sc", name="sc")
                    sc_tiles[k] = sc
                    for jj in range(2):
                        jt = 2 * p + jj
                        nc.tensor.matmul(
                            out=sc[:, jj, :],
                            lhsT=qt_sb[hp, 1, jt * JT : (jt + 1) * JT],
                            rhs=qt_sb[hp, 0, i0 : i0 + ich],
                            start=True,
                            stop=True,
                        )

                def emit_exp(k):
                    p, h = steps[k]
                    sc = sc_tiles.pop(k)
                    ex = expp.tile([128, 2, IH], BF16, tag="ex", name="ex")
                    ex_tiles[k] = ex
                    nc.scalar.activation(
                        out=ex, in_=sc, func=EXP, scale=1.0 / np.sqrt(DK)
                    )
                    for jj in range(2):
                        jt = 2 * p + jj
                        off = jt * JT - i0
                        if off >= 0:  # diagonal tile: causal mask
                            if off > 0:
                                nc.vector.memset(ex[:, jj, 0:off], 0.0)
                            nc.vector.tensor_mul(
                                ex[:, jj, off : off + JT],
                                ex[:, jj, off : off + JT],
                                tri_sb,
                            )

                def emit_pv(k):
                    p, h = steps[k]
                    ex = ex_tiles.pop(k)
                    for jj in range(2):
                        jt = 2 * p + jj
                        nc.tensor.matmul(
                            out=pv[h][0 : DK + 1, :],
                            lhsT=vp_sb[:, jt, h * (DK + 1) : (h + 1) * (DK + 1)],
                            rhs=ex[:, jj, :],
                            start=(jt == 0),
                            stop=(jt == njt - 1),
                        )

                def drain(budget, frac):
                    while budget > 0:
                        eligible = [e for e in pending if e[0] <= frac]
                        if not eligible:
                            return
                        progressed = False
                        for e in eligible:
                            if budget <= 0:
                                return
                            try:
                                next(e[1])
                                budget -= 1
                                progressed = True
                            except StopIteration:
                                pending.remove(e)
                        if not progressed:
                            return

                total_ops = sum(
                    34 if e[0] == 0.0 else 14 for e in pending
                )
                emit_sc(0)
                for k in range(nsteps):
                    if k + 1 < nsteps:
                        emit_sc(k + 1)
                    emit_exp(k)
                    if k - 2 >= 0:
                        emit_pv(k - 2)
                    drain(
                        (total_ops + nsteps - 1 - k) // nsteps + 1,
                        (k + 1) / nsteps,
                    )
                emit_pv(nsteps - 2)
                emit_pv(nsteps - 1)
                return pv

            # ---- main schedule ----
            prefetch_xt(0)
            if nch > 1:
                prefetch_xt(1)
            for _ in gen_proj(0):
                pass
            pending = []
            for c in range(nch):
                if c + 2 < nch:
                    prefetch_xt(c + 2)
                if c + 1 < nch:
                    pending.append([0.0, gen_proj(c + 1)])
                pv = emit_chunk(c, pending)
                out_slot = {}
                pending.append([0.0, gen_tail_head(c, pv, out_slot)])
                pending.append([0.55, gen_tail_y(c, out_slot)])
            for _, g in pending:
                for _ in g:
                    pass
    nc.compile()
    return nc


def get_nc(T):
    if T not in _NC_CACHE:
        _NC_CACHE[T] = build(T)
    return _NC_CACHE[T]


TRI = np.triu(np.ones((JT, JT))).astype(NPBF)  # 1 where key j <= query i
IDENT = np.eye(128).astype(NPBF)

LAST_RESULTS = None  # BassKernelResults of the last run (for profiling)


def _tile_dk(w):
    """[D, C] -> [128, D//128, C] partition-tiled so the device DMA is
    contiguous: out[p, t, c] = w[t*128 + p, c]."""
    Dd, C = w.shape
    return np.ascontiguousarray(
        w.reshape(Dd // 128, 128, C).transpose(1, 0, 2)
    )


def make_in_maps(x, Wq, Wk, Wv, Wo, n_cores=8):
    """x: (T, D) fp32. Returns per-core input maps (bf16 operands)."""
    xT = _tile_dk(np.ascontiguousarray(x.T).astype(NPBF))
    maps = []
    for n in range(n_cores):
        sl = slice(CPC * n, CPC * (n + 1))
        maps.append(
            {
                "xT": xT,
                "wqT": _tile_dk(Wq[sl, :].T.astype(NPBF)),
                "wkT": _tile_dk(Wk[sl, :].T.astype(NPBF)),
                "wvT": _tile_dk(Wv[sl, :].T.astype(NPBF)),
                "woT": np.ascontiguousarray(Wo[:, sl].T).astype(NPBF),
                "tri": TRI,
                "ident": IDENT,
            }
        )
    return maps


def run(x, Wq, Wk, Wv, Wo, T=None, n_cores=8, trace=False):
    global LAST_RESULTS
    T = T if T is not None else x.shape[0]
    nc = get_nc(T)
    in_maps = make_in_maps(x, Wq, Wk, Wv, Wo, n_cores)
    res = run_bass_kernel_spmd(
        nc, in_maps, core_ids=list(range(n_cores)), trace=trace
    )
    LAST_RESULTS = res
    y = np.zeros((T, D), dtype=np.float64)
    for r in res.results:
        y += r["y"].astype(np.float64)
    return y.astype(np.float32)


def kernel(x, Wq, Wk, Wv, Wo):
    x = np.asarray(x, dtype=np.float32)
    B, T, _ = x.shape
    trace = bool(os.environ.get("MHA_TRACE"))
    y = run(
        np.ascontiguousarray(x.reshape(T, D)),
        np.asarray(Wq, np.float32),
        np.asarray(Wk, np.float32),
        np.asarray(Wv, np.float32),
        np.asarray(Wo, np.float32),
        T=T,
        trace=trace,
    )
    if trace and LAST_RESULTS is not None and LAST_RESULTS.exec_time_ns:
        print(f"HW exec time: {LAST_RESULTS.exec_time_ns} ns")
    return y.reshape(B, T, D)


# revision 23
# speedup vs baseline: 1.4608x; 1.1570x over previous
# Multi-head causal attention (B=1, T=4096, D=1024, H=16) on 8 TRN2 NeuronCores.
#
# Sharding: tensor-parallel over heads. Core n computes head channels
# [128n, 128n+128) (= heads 2n, 2n+1), runs the full causal attention for its
# two heads, and produces a full-shape partial output
#   y_n = attn_out[:, ch_n] @ Wo[:, ch_n].T        (4096, 1024)
# The host sums the 8 partials (row-sharded Wo contraction) — no collectives.
#
# Device-side layout (per core):
#   xT   [128, 8, T]   x transposed + partition-tiled on the host so the DMA
#                      is contiguous; contraction (d_model) on partitions.
#   QT/KT [128, 2, T]  head channels on partitions (h0: 0-63, h1: 64-127), bf16
#   scoresT[j, i]      keys on partitions, queries on free dim; the softmax sum
#                      over keys rides the PV matmul via a ones-column appended
#                      to V (V' = [V | 1], M=65): psum row 64 = rowsum.
#   exp on ScalarE directly PSUM->SBUF (bf16 out) with 1/sqrt(dk) folded into
#   the activation scale; psum accumulation is always fp32.
#   Causal: only key tiles with j <= i are computed; diagonal 128x128 blocks
#   are masked by a 0/1 upper-triangular multiply after exp.
#
# Schedule: the ScalarE exp stream (144 ACTIVATEs of 128x1024, ~1.15us each)
# is the critical resource; everything is arranged to keep it saturated.
# Attention is a flat sequence of (key-tile-pair, head) steps per 512-query
# chunk; at step k the kernel emits score matmuls for step k+1, the exp for
# step k, and PV matmuls for step k-2 (the 2-step lag keeps psum-handoff
# waits at chunk boundaries off the in-order PE queue's critical path).
# PSUM banks: 2 rotating 2-bank score tiles (4), one 2-bank aux slot for
# projection/output-projection tiles (2), two 1-bank PV accumulators (2).
# The Q/K/V projection of chunk c+1 and the normalization+output projection
# tail of chunk c-1 are emitted interleaved into chunk c's attention steps;
# the tail's PE section (y matmuls) is gated to the last 45% of the chunk so
# the rowsum-reciprocal DMA roundtrip it depends on has completed.

import os
import sys

for _p in ("/opt/trn_rl_repo", "/root/.axon_site/_ro/trn_rl_repo"):
    if os.path.isdir(_p) and _p not in sys.path:
        sys.path.insert(0, _p)

import ml_dtypes
import numpy as np

def _ensure_axon_ntff_hook():
    """The agent image's antenv package lacks axon_hooks, which makes
    run_bass_kernel_spmd(trace=True) crash at import under axon. Provide the
    module and register the boot hook so NTFF profiling works."""
    import types

    try:
        import antenv.axon_hooks  # noqa: F401
        return
    except ImportError:
        pass
    try:
        import antenv
    except ImportError:
        return
    mod = types.ModuleType("antenv.axon_hooks")
    mod._hook = None
    mod.set_axon_ntff_profile_hook = lambda h: setattr(mod, "_hook", h)
    mod.get_axon_ntff_profile_hook = lambda: mod._hook
    sys.modules["antenv.axon_hooks"] = mod
    antenv.axon_hooks = mod
    try:
        from trn_agent_boot.trn_boot import _ntff_profile_via_ctypes

        so = "/opt/axon/libaxon_pjrt.so"
        if os.path.exists(so):
            mod._hook = _ntff_profile_via_ctypes(so)
    except Exception:
        pass


_ensure_axon_ntff_hook()

import concourse.bass as bass
import concourse.tile as tile
from concourse import bacc
from concourse import mybir
from concourse.bass_utils import run_bass_kernel_spmd

F32 = mybir.dt.float32
BF16 = mybir.dt.bfloat16
EXP = mybir.ActivationFunctionType.Exp
NPBF = ml_dtypes.bfloat16

D = 1024          # d_model
DK = 64           # head dim
CPC = 128         # channels per core (2 heads)
ICH = 512         # query-chunk size
IH = 512          # i-half width (matmul N / psum bank limit)
JT = 128          # key-tile size
NT = D // 128     # d_model tiles

_NC_CACHE = {}


def build(T):
    """Build the per-core Bass program for sequence length T."""
    nc = bacc.Bacc(None, target_bir_lowering=False, debug=False)
    ich = min(ICH, T)
    nch = T // ich

    xT_d = nc.dram_tensor(
        "xT", [T // ICH if T >= ICH else 1, 128, NT, min(ICH, T)], BF16,
        kind="ExternalInput",
    )
    wqT_d = nc.dram_tensor("wqT", [128, NT, CPC], BF16, kind="ExternalInput")
    wkT_d = nc.dram_tensor("wkT", [128, NT, CPC], BF16, kind="ExternalInput")
    wvT_d = nc.dram_tensor("wvT", [128, NT, CPC], BF16, kind="ExternalInput")
    woT_d = nc.dram_tensor("woT", [CPC, D], BF16, kind="ExternalInput")
    tri_d = nc.dram_tensor("tri", [JT, JT], BF16, kind="ExternalInput")
    ident_d = nc.dram_tensor("ident", [128, 128], BF16, kind="ExternalInput")
    y_d = nc.dram_tensor("y", [T, D], F32, kind="ExternalOutput")
    rs_scratch = nc.dram_tensor("rs_scratch", [nch, 2, ich], F32)
    rs2_scratch = nc.dram_tensor("rs2_scratch", [nch, 2 * ich], F32)

    with tile.TileContext(nc) as tc:
        with (
            tc.tile_pool(name="const", bufs=1) as const,
            tc.tile_pool(name="xtp", bufs=2) as xtp,
            tc.tile_pool(name="vtp", bufs=2) as vtp,
            tc.tile_pool(name="expp", bufs=6) as expp,
            tc.tile_pool(name="outp", bufs=2) as outp,
            tc.tile_pool(name="yp", bufs=4) as yp,
            tc.tile_pool(name="psp", bufs=2, space="PSUM") as psp,
            tc.tile_pool(name="pvp", bufs=1, space="PSUM") as pvp,
        ):
            xt_tiles = {}

            def prefetch_xt(c):
                xt_ch = xtp.tile([128, NT, ich], BF16, tag="xt", name="xt_ch")
                nc.sync.dma_start(out=xt_ch, in_=xT_d[c])
                xt_tiles[c] = xt_ch

            # x chunk 0 first — it gates the first projection matmuls
            prefetch_xt(0)
            if nch > 1:
                prefetch_xt(1)

            # ---- warm the ScalarE exp table while the weight DMAs run ----
            warm = const.tile([128, 1], F32)
            nc.vector.memset(warm, 0.0)
            nc.scalar.activation(out=warm, in_=warm, func=EXP)

            # ---- constants / persistent state ----
            wq_sb = const.tile([128, NT, 128], BF16)
            wk_sb = const.tile([128, NT, 128], BF16)
            wv_sb = const.tile([128, NT, 128], BF16)
            for w_sb, w_d in ((wq_sb, wqT_d), (wk_sb, wkT_d), (wv_sb, wvT_d)):
                nc.sync.dma_start(out=w_sb, in_=w_d[:, :, :])
            wo_sb = const.tile([128, D], BF16)
            nc.sync.dma_start(out=wo_sb, in_=woT_d[:, :])
            tri_sb = const.tile([JT, JT], BF16)
            nc.sync.dma_start(out=tri_sb, in_=tri_d[:, :])
            id_sb = const.tile([128, 128], BF16)
            nc.sync.dma_start(out=id_sb, in_=ident_d[:, :])

            qt_sb = const.tile([128, 2, T], BF16)  # [:,0,:]=QT, [:,1,:]=KT
            # V' = [V_h | 1] per head: [j, jt, 2*65]
            vp_sb = const.tile([128, T // JT, 2 * (DK + 1)], BF16)
            ones_view = vp_sb.rearrange("p t (h c) -> p t h c", h=2)[
                :, :, :, DK : DK + 1
            ]
            nc.vector.memset(ones_view, 1.0)

            def gen_proj(c):
                """Generator emitting the Q/K/V projection for chunk c,
                one PE/DVE op per yield (the x chunk was prefetched).
                NOTE: must be fully emitted before chunk c's attention —
                the vp_sb copy's rearranged write is not reliably ordered
                against the PV matmuls by the dependency tracker."""
                i0 = c * ich
                xt_ch = xt_tiles.pop(c)
                qk_ps = psp.tile([128, 2, ich], F32, tag="aux", bufs=1, name="qk_ps")
                for qk, w_sb in ((0, wq_sb), (1, wk_sb)):
                    for t in range(NT):
                        nc.tensor.matmul(
                            out=qk_ps[:, qk, :],
                            lhsT=w_sb[:, t, :],
                            rhs=xt_ch[:, t, :],
                            start=(t == 0),
                            stop=(t == NT - 1),
                        )
                        yield
                nc.vector.tensor_copy(out=qt_sb[:, :, i0 : i0 + ich], in_=qk_ps)
                yield
                vt_ps = psp.tile([128, 2, ich], F32, tag="aux", bufs=1, name="vt_ps")
                for t in range(NT):
                    nc.tensor.matmul(
                        out=vt_ps[:, 0, :],
                        lhsT=wv_sb[:, t, :],
                        rhs=xt_ch[:, t, :],
                        start=(t == 0),
                        stop=(t == NT - 1),
                    )
                    yield
                vt_sb = vtp.tile([128, ich], BF16, tag="vt", name="vt_sb")
                nc.vector.tensor_copy(out=vt_sb, in_=vt_ps[:, 0, :])
                yield
                vn_ps = psp.tile(
                    [128, ich // 128, 128], BF16, tag="aux", bufs=1, name="vn_ps"
                )
                for sdx in range(ich // 128):
                    nc.tensor.transpose(
                        out=vn_ps[:, sdx, :],
                        in_=vt_sb[:, sdx * 128 : (sdx + 1) * 128],
                        identity=id_sb,
                    )
                    yield
                jt0 = i0 // JT
                nc.vector.tensor_copy(
                    out=vp_sb.rearrange("p t (h c) -> p t h c", h=2)[
                        :, jt0 : jt0 + ich // 128, :, 0:DK
                    ],
                    in_=vn_ps.rearrange("p s (h c) -> p s h c", h=2),
                )
                yield

            def gen_tail_head(c, pv, out_slot):
                """Rowsum DMA straight out of psum (starts the reciprocal
                roundtrip immediately), raw-output extraction (frees the pv
                psum banks), then the reciprocal on the partition-scattered
                layout (a 1-partition reciprocal would cost 3.3us) and the
                broadcast back."""
                rs_sb = outp.tile([1, 2, ich], F32, tag="rs", name="rs_sb")
                for h in range(2):
                    nc.vector.tensor_copy(
                        out=rs_sb[0:1, h, :], in_=pv[h][DK : DK + 1, :]
                    )
                nc.sync.dma_start(out=rs_scratch[c : c + 1, :, :], in_=rs_sb)
                outt = outp.tile([128, ich], BF16, tag="outt", name="outt")
                for h in range(2):
                    nc.vector.tensor_copy(
                        out=outt[h * DK : (h + 1) * DK, :], in_=pv[h][0:DK, :]
                    )
                out_slot["outt"] = outt
                yield
                rsp = outp.tile([128, 2 * ich // 128], F32, tag="rsp", name="rsp")
                nc.sync.dma_start(
                    out=rsp,
                    in_=rs_scratch[c].rearrange("h i -> (h i)").rearrange(
                        "(p f) -> p f", p=128
                    ),
                )
                yield
                nc.vector.reciprocal(out=rsp, in_=rsp)
                yield
                nc.sync.dma_start(
                    out=rs2_scratch[c].rearrange("(p f) -> p f", p=128), in_=rsp
                )
                yield
                bc = outp.tile([128, ich], F32, tag="bc", name="bc")
                for h in range(2):
                    nc.gpsimd.dma_start(
                        out=bc[h * DK : (h + 1) * DK, :],
                        in_=rs2_scratch[c].rearrange("(h i) -> h i", h=2)[
                            h : h + 1, :
                        ].to_broadcast([DK, ich]),
                    )
                    yield
                out_slot["bc"] = bc

            def gen_tail_y(c, out_slot, last=False):
                """Normalize and project: gated to late in the next chunk so
                the y matmuls never head-of-line block the PE queue on the
                reciprocal roundtrip. y stores DMA straight from psum (no
                DVE copy). The last chunk's y tiles use the score rotation
                (free by then) so its stores pipeline 2-deep."""
                i0 = c * ich
                outt, bc = out_slot["outt"], out_slot["bc"]
                nc.vector.tensor_mul(outt, outt, bc)
                yield
                for sidx in range(ich // 128):
                    y_ps = psp.tile(
                        [128, 2, IH], F32,
                        tag="sc" if last else "aux",
                        bufs=2 if last else 1,
                        name="y_ps",
                    )
                    for e in range(2):
                        nc.tensor.matmul(
                            out=y_ps[:, e, :],
                            lhsT=outt[:, sidx * 128 : (sidx + 1) * 128],
                            rhs=wo_sb[:, e * IH : (e + 1) * IH],
                            start=True,
                            stop=True,
                        )
                        yield
                    y_sb = yp.tile([128, D], F32, tag="y", name="y_sb")
                    nc.vector.tensor_copy(
                        out=y_sb, in_=y_ps.rearrange("p a b -> p (a b)")
                    )
                    yield
                    r0 = i0 + sidx * 128
                    nc.sync.dma_start(out=y_d[r0 : r0 + 128, :], in_=y_sb)
                    yield

            def emit_chunk(c, pending):
                """Attention steps for chunk c, draining `pending` generator
                entries [min_frac, gen] into the step slack."""
                i0 = c * ich
                njt = (i0 + ich) // JT
                pv = [
                    pvp.tile([128, ich], F32, tag="pv0", name="pv0"),
                    pvp.tile([128, ich], F32, tag="pv1", name="pv1"),
                ]
                # one step per 128-key tile; both heads' K=64 score matmuls
                # are row-tiled (tile_position auto-derives from the lhsT/out
                # base partitions: h0 rows 0-63, h1 rows 64-127) and execute
                # CONCURRENTLY in the PE array — one N=512 window for both
                nsteps = njt
                sc_tiles = {}
                ex_tiles = {}

                def emit_sc(k):
                    sc = psp.tile([128, 2, IH], F32, tag="sc", name="sc")
                    sc_tiles[k] = sc
                    for h in range(2):
                        hp = slice(h * DK, (h + 1) * DK)
                        nc.tensor.matmul(
                            out=sc[:, h, :],
                            lhsT=qt_sb[hp, 1, k * JT : (k + 1) * JT],
                            rhs=qt_sb[hp, 0, i0 : i0 + ich],
                            start=True,
                            stop=True,
                        )

                def emit_exp(k):
                    sc = sc_tiles.pop(k)
                    ex = expp.tile([128, 2, IH], BF16, tag="ex", name="ex")
                    ex_tiles[k] = ex
                    nc.scalar.activation(
                        out=ex, in_=sc, func=EXP, scale=1.0 / np.sqrt(DK)
                    )
                    off = k * JT - i0
                    if off >= 0:  # diagonal tile: causal mask
                        if off > 0:
                            nc.vector.memset(ex[:, :, 0:off], 0.0)
                        for h in range(2):
                            nc.vector.tensor_mul(
                                ex[:, h, off : off + JT],
                                ex[:, h, off : off + JT],
                                tri_sb,
                            )

                def emit_pv(k):
                    ex = ex_tiles.pop(k)
                    for h in range(2):
                        nc.tensor.matmul(
                            out=pv[h][0 : DK + 1, :],
                            lhsT=vp_sb[:, k, h * (DK + 1) : (h + 1) * (DK + 1)],
                            rhs=ex[:, h, :],
                            start=(k == 0),
                            stop=(k == njt - 1),
                        )

                def drain(budget, frac):
                    while budget > 0:
                        eligible = [e for e in pending if e[0] <= frac]
                        if not eligible:
                            return
                        progressed = False
                        for e in eligible:
                            if budget <= 0:
                                return
                            try:
                                next(e[1])
                                budget -= 1
                                progressed = True
                            except StopIteration:
                                pending.remove(e)
                        if not progressed:
                            return

                total_ops = sum(
                    20 if e[0] == 0.0 else 14 for e in pending
                )
                emit_sc(0)
                for k in range(nsteps):
                    if k + 1 < nsteps:
                        emit_sc(k + 1)
                    emit_exp(k)
                    if k - 2 >= 0:
                        emit_pv(k - 2)
                    drain(
                        (total_ops + nsteps - 1 - k) // nsteps + 1,
                        (k + 1) / nsteps,
                    )
                emit_pv(nsteps - 2)
                emit_pv(nsteps - 1)
                return pv

            # ---- main schedule ----
            for _ in gen_proj(0):
                pass
            pending = []
            for c in range(nch):
                if c + 2 < nch:
                    prefetch_xt(c + 2)
                if c + 1 < nch:
                    pending.append([0.0, gen_proj(c + 1)])
                pv = emit_chunk(c, pending)
                out_slot = {}
                pending.append([0.0, gen_tail_head(c, pv, out_slot)])
                pending.append(
                    [0.55, gen_tail_y(c, out_slot, last=(c == nch - 1))]
                )
            for _, g in pending:
                for _ in g:
                    pass
    nc.compile()
    return nc


def get_nc(T):
    if T not in _NC_CACHE:
        _NC_CACHE[T] = build(T)
    return _NC_CACHE[T]


TRI = np.triu(np.ones((JT, JT))).astype(NPBF)  # 1 where key j <= query i
IDENT = np.eye(128).astype(NPBF)

LAST_RESULTS = None  # BassKernelResults of the last run (for profiling)


def _tile_dk(w):
    """[D, C] -> [128, D//128, C] partition-tiled so the device DMA is
    contiguous: out[p, t, c] = w[t*128 + p, c]."""
    Dd, C = w.shape
    return np.ascontiguousarray(
        w.reshape(Dd // 128, 128, C).transpose(1, 0, 2)
    )


def make_in_maps(x, Wq, Wk, Wv, Wo, n_cores=8):
    """x: (T, D) fp32. Returns per-core input maps (bf16 operands)."""
    T = x.shape[0]
    ich = min(ICH, T)
    # [nch, 128, NT, ich]: chunk-contiguous so each chunk is one linear DMA
    xT = np.ascontiguousarray(
        x.T.astype(NPBF)
        .reshape(NT, 128, T // ich, ich)
        .transpose(2, 1, 0, 3)
    )
    maps = []
    for n in range(n_cores):
        sl = slice(CPC * n, CPC * (n + 1))
        maps.append(
            {
                "xT": xT,
                "wqT": _tile_dk(Wq[sl, :].T.astype(NPBF)),
                "wkT": _tile_dk(Wk[sl, :].T.astype(NPBF)),
                "wvT": _tile_dk(Wv[sl, :].T.astype(NPBF)),
                "woT": np.ascontiguousarray(Wo[:, sl].T).astype(NPBF),
                "tri": TRI,
                "ident": IDENT,
            }
        )
    return maps


def run(x, Wq, Wk, Wv, Wo, T=None, n_cores=8, trace=False):
    global LAST_RESULTS
    T = T if T is not None else x.shape[0]
    nc = get_nc(T)
    in_maps = make_in_maps(x, Wq, Wk, Wv, Wo, n_cores)
    res = run_bass_kernel_spmd(
        nc, in_maps, core_ids=list(range(n_cores)), trace=trace
    )
    LAST_RESULTS = res
    y = np.zeros((T, D), dtype=np.float64)
    for r in res.results:
        y += r["y"].astype(np.float64)
    return y.astype(np.float32)


def kernel(x, Wq, Wk, Wv, Wo):
    x = np.asarray(x, dtype=np.float32)
    B, T, _ = x.shape
    trace = bool(os.environ.get("MHA_TRACE"))
    y = run(
        np.ascontiguousarray(x.reshape(T, D)),
        np.asarray(Wq, np.float32),
        np.asarray(Wk, np.float32),
        np.asarray(Wv, np.float32),
        np.asarray(Wo, np.float32),
        T=T,
        trace=trace,
    )
    if trace and LAST_RESULTS is not None and LAST_RESULTS.exec_time_ns:
        print(f"HW exec time: {LAST_RESULTS.exec_time_ns} ns")
    return y.reshape(B, T, D)
